# revision 2
# baseline (speedup 1.0000x reference)
"""Trainium2 Bass kernel for nn_DiscreteAutoregressiveFlow (sampling, forward).

Math: `inputs` is an exact one-hot [B, L, V] tensor. For a row holding token v
the reference reduces to out_row = one_hot((scale_tok[v]*v + loc_tok[v]) % V),
or the zero row when scale_tok[v] == 0, where loc_tok/scale_tok are argmaxes
of W[v]+b halves (host-precomputable from W/b alone). So the whole flow is a
fixed linear map applied per row, which TensorE evaluates as a matmul against
a host-built table. All products/sums are exact in fp8/f32.

Device pipeline (memory-bound streaming):
  - Host pre-transposes x to xt [128, 8192] fp8 per core: partition = 64*half
    + v, column = row-within-half. One-hots are exact in fp8_e4m3.
  - The table encodes the OUTPUT ONE-HOT AS AN 8-BYTE BITMASK: lhsT[v, j]
    holds float(1 << (u%8)) at j = u//8 for u = cmap[v] (values {1..128},
    exact in fp8e4m3). Each matmul therefore emits [32, 512] (8 byte-lanes x
    2 halves + 16 zero-pad lanes) instead of [128, 512]: 4 matmuls pack one
    PSUM bank at tile_position col offsets 0/32/64/96, so one PSUM->SBUF
    cast-copy drains FOUR matmuls (4 copies total instead of 16) and the
    output DMA shrinks to 256 KB/core. Host np.unpackbits restores one-hots.
  - 16 matmuls of N=512 over a once-loaded stationary table; 8192 PE column
    cycles per core.

Scheduling: each HWDGE dma_start costs ~650ns of SERIAL sequencer time
(DIRECT2D descriptor generation) regardless of size, so DMAs are few and
large, with input dispatch split across both HWDGE rings (SP + ACT) and a
small head chunk so TensorE starts early.
Sharding: pure data parallel over B*L rows, 8 cores, no collectives.
"""

import numpy as np

V = 64
P = 128
N_CORES = 8
B, L = 16, 8192
ROWS = B * L                      # 131072
ROWS_PER_CORE = ROWS // N_CORES   # 16384
HALF = ROWS_PER_CORE // 2         # 8192 columns per core (2 rows per column)
MM_N = 512                        # PSUM bank = 512 f32
N_WIN = HALF // MM_N              # 16 matmul windows
N_BANKS = N_WIN // 4              # 4 windows packed per PSUM bank
OUT_W = N_BANKS * MM_N            # 2048 output columns (byte-packed)

# Input chunks (width, ring), tapered and interleaved across both HWDGE
# rings; the stationary table rides the GPSIMD (SWDGE) queue so it does not
# consume a serial HWDGE dispatch slot.
IN_CHUNKS = ((512, "sp"), (512, "sp"), (2048, "act"), (2048, "sp"),
             (1536, "act"), (1024, "sp"), (512, "sp"))
assert sum(w for w, _ in IN_CHUNKS) == HALF
assert all(w % MM_N == 0 for w, _ in IN_CHUNKS)

_CACHE = {}


def _build_nc(in_chunks=IN_CHUNKS):
    import concourse.bacc as bacc
    import concourse.mybir as mybir
    from concourse.tile import TileContext

    f32 = mybir.dt.float32
    fp8 = mybir.dt.float8e4

    nc = bacc.Bacc("TRN2", target_bir_lowering=False, name="daf_mm")
    xt = nc.dram_tensor("xt", [P, HALF], fp8, kind="ExternalInput")
    mt = nc.dram_tensor("mt", [P, 32], fp8, kind="ExternalInput")
    yt = nc.dram_tensor("yt", [P, OUT_W], fp8, kind="ExternalOutput")

    with TileContext(nc) as tc:
        with (
            tc.tile_pool(name="const", bufs=1) as constp,
            tc.tile_pool(name="io", bufs=1) as iop,
            tc.tile_pool(name="ps", bufs=1, space="PSUM") as psp,
        ):
            # Stationary byte-table: tiny, first dispatch on the ACT ring.
            mt_st = constp.tile([P, 32], fp8, tag="mt")
            nc.scalar.dma_start(mt_st[:], mt[:])

            # All input DMAs dispatched next, alternating rings.
            in_tiles = []   # (tile, col_start, width)
            cs = 0
            for ci, (cw, q) in enumerate(in_chunks):
                xtile = iop.tile([P, cw], fp8, tag=f"x{ci}")
                eng = {"sp": nc.sync, "act": nc.scalar, "gp": nc.gpsimd}[q]
                eng.dma_start(xtile[:], xt[:][:, cs : cs + cw])
                in_tiles.append((xtile, cs, cw))
                cs += cw

            def rhs_window(w):
                col = w * MM_N
                for xtile, xcs, xcw in in_tiles:
                    if xcs <= col and col + MM_N <= xcs + xcw:
                        return xtile[:][:, col - xcs : col - xcs + MM_N]
                raise AssertionError(w)

            ps_tiles = [
                psp.tile([P, MM_N], f32, tag=f"ps{b}", name=f"ps{b}")
                for b in range(N_BANKS)
            ]
            o01 = iop.tile([P, 2 * MM_N], fp8, tag="o01")
            o2 = iop.tile([P, MM_N], fp8, tag="o2")
            o3a = iop.tile([P, MM_N // 2], fp8, tag="o3a")
            o3b = iop.tile([P, MM_N // 2], fp8, tag="o3b")
            for w in range(N_WIN):
                b, s = w // 4, w % 4
                ps = ps_tiles[b]
                nc.tensor.matmul(
                    ps[:][32 * s : 32 * s + 32, :],
                    mt_st[:],
                    rhs_window(w),
                    # Every matmul is its own "group": start=True clears the
                    # bank's has_written bits (data of other strips persists)
                    # and overwrites this strip; start=False would accumulate
                    # onto stale PSUM state from a previous NEFF execution.
                    start=True,
                    stop=True,
                    tile_position=(0, 32 * s),
                    skip_group_check=True,
                )
                if s == 3:
                    # Copies balance DVE/ACT by bank readiness; the final
                    # bank splits into two fully parallel paths (separate
                    # tiles, engines, rings) to shorten the tail.
                    if b == 0:
                        nc.vector.tensor_copy(o01[:][:, :MM_N], ps[:])
                    elif b == 1:
                        nc.scalar.copy(o01[:][:, MM_N:], ps[:])
                        nc.sync.dma_start(yt[:][:, 0 : 2 * MM_N], o01[:])
                    elif b == 2:
                        nc.vector.tensor_copy(o2[:], ps[:])
                        nc.sync.dma_start(
                            yt[:][:, 2 * MM_N : 3 * MM_N], o2[:]
                        )
                    else:
                        half = MM_N // 2
                        base = 3 * MM_N
                        nc.vector.tensor_copy(o3a[:], ps[:][:, :half])
                        nc.sync.dma_start(
                            yt[:][:, base : base + half], o3a[:]
                        )
                        nc.scalar.copy(o3b[:], ps[:][:, half:])
                        nc.scalar.dma_start(
                            yt[:][:, base + half : base + MM_N], o3b[:]
                        )

    nc.finalize()
    return nc


def _get_nc(in_chunks=IN_CHUNKS):
    key = in_chunks
    if key not in _CACHE:
        _CACHE[key] = _build_nc(in_chunks)
    return _CACHE[key]


def _host_mtab(W: np.ndarray, b: np.ndarray) -> np.ndarray:
    """[128, 32] fp8 byte-table: lanes 0-7 halfA, 8-15 halfB, 16-31 zero."""
    import ml_dtypes

    net = W.astype(np.float32) + b.astype(np.float32)[None, :]   # [V, 2V]
    loc_tok = np.argmax(net[:, :V], axis=1)
    scale_tok = np.argmax(net[:, V:], axis=1)
    tgt = (scale_tok * np.arange(V, dtype=np.int64) + loc_tok) % V
    bt = np.zeros((V, 8), dtype=np.float32)
    nz = np.flatnonzero(scale_tok != 0)
    bt[nz, tgt[nz] // 8] = (1 << (tgt[nz] % 8)).astype(np.float32)
    mt = np.zeros((P, 32), dtype=np.float32)
    mt[:V, :8] = bt
    mt[V:, 8:16] = bt
    return mt.astype(ml_dtypes.float8_e4m3)


def _host_in_maps(inputs: np.ndarray, W: np.ndarray, b: np.ndarray):
    import ml_dtypes

    x8 = inputs.reshape(ROWS, V).astype(ml_dtypes.float8_e4m3)
    # [core, half, row, v] -> [core, half, v, row] -> [core, 128, HALF]
    xt = np.ascontiguousarray(
        x8.reshape(N_CORES, 2, HALF, V).transpose(0, 1, 3, 2)
    ).reshape(N_CORES, P, HALF)
    mt = _host_mtab(W, b)
    return [{"xt": xt[c], "mt": mt} for c in range(N_CORES)]


def _host_gather(results, shape, dtype) -> np.ndarray:
    import ml_dtypes

    yt = np.stack([np.asarray(r["yt"]) for r in results])   # [8, 128, 2048] fp8
    # fp8 -> uint8 integer values (all entries are {0, 1, 2, ..., 128}: exact)
    lut = np.arange(256, dtype=np.uint8).view(ml_dtypes.float8_e4m3)
    lut = lut.astype(np.float32)
    lut[~np.isfinite(lut)] = 0.0
    lut = np.clip(lut, 0, 255).astype(np.uint8)
    ytu = lut[yt.view(np.uint8)]                            # [8, 128, 2048] u8
    v5 = ytu.reshape(N_CORES, 4, 32, N_BANKS, MM_N)         # [c, s, m, b, nl]
    bytes_ = v5[:, :, :16]                                  # [c, s, 16, b, nl]
    bytes_ = bytes_.transpose(0, 2, 3, 1, 4)                # [c, hj, b, s, nl]
    bytes_ = bytes_.reshape(N_CORES, 2, 8, N_WIN, MM_N)     # [c, h, j, w, nl]
    bytes_ = np.ascontiguousarray(
        bytes_.transpose(0, 1, 3, 4, 2)                     # [c, h, w, nl, j]
    ).reshape(N_CORES, 2, HALF, 8)
    bits = np.unpackbits(bytes_, axis=-1, bitorder="little")
    y = bits.reshape(ROWS, V).astype(np.float32)
    return y.reshape(shape).astype(dtype, copy=False)


def kernel(inputs: np.ndarray, W: np.ndarray, b: np.ndarray) -> np.ndarray:
    from concourse import bass_utils

    in_maps = _host_in_maps(np.asarray(inputs), np.asarray(W), np.asarray(b))
    nc = _get_nc()
    res = bass_utils.run_bass_kernel_spmd(nc, in_maps, core_ids=list(range(N_CORES)))
    return _host_gather(res.results, inputs.shape, inputs.dtype)


# revision 4
# speedup vs baseline: 1.0659x; 1.0659x over previous
"""Trainium2 Bass kernel for nn_DiscreteAutoregressiveFlow (sampling, forward).

Math: `inputs` is an exact one-hot [B, L, V] tensor. For a row holding token v
the reference reduces to out_row = one_hot((scale_tok[v]*v + loc_tok[v]) % V),
or the zero row when scale_tok[v] == 0, where loc_tok/scale_tok are argmaxes
of W[v]+b halves (host-precomputable from W/b alone). So the whole flow is a
fixed linear map applied per row, which TensorE evaluates as a matmul against
a host-built table. All products/sums are exact in fp8/f32.

Device pipeline (memory-bound streaming):
  - Host pre-transposes x to xt [128, 8192] fp8 per core: partition = 64*half
    + v, column = row-within-half. One-hots are exact in fp8_e4m3.
  - The table encodes the OUTPUT ONE-HOT AS AN 8-BYTE BITMASK: lhsT[v, j]
    holds float(1 << (u%8)) at j = u//8 for u = cmap[v] (values {1..128},
    exact in fp8e4m3). Each matmul therefore emits [32, 512] (8 byte-lanes x
    2 halves + 16 zero-pad lanes) instead of [128, 512]: 4 matmuls pack one
    PSUM bank at tile_position col offsets 0/32/64/96, so one PSUM->SBUF
    cast-copy drains FOUR matmuls (4 copies total instead of 16) and the
    output DMA shrinks to 256 KB/core. Host np.unpackbits restores one-hots.
  - 16 matmuls of N=512 over a once-loaded stationary table; 8192 PE column
    cycles per core.

Scheduling: each HWDGE dma_start costs ~650ns of SERIAL sequencer time
(DIRECT2D descriptor generation) regardless of size, so DMAs are few and
large, with input dispatch split across both HWDGE rings (SP + ACT) and a
small head chunk so TensorE starts early.
Sharding: pure data parallel over B*L rows, 8 cores, no collectives.
"""

import numpy as np

V = 64
P = 128
N_CORES = 8
B, L = 16, 8192
ROWS = B * L                      # 131072
ROWS_PER_CORE = ROWS // N_CORES   # 16384
HALF = ROWS_PER_CORE // 2         # 8192 columns per core (2 rows per column)
MM_N = 512                        # PSUM bank = 512 f32
N_WIN = HALF // MM_N              # 16 matmul windows
N_BANKS = N_WIN // 4              # 4 windows packed per PSUM bank
OUT_W = N_BANKS * MM_N            # 2048 output columns (byte-packed)

# Input chunks (width, ring), tapered and interleaved across both HWDGE
# rings; the stationary table rides the GPSIMD (SWDGE) queue so it does not
# consume a serial HWDGE dispatch slot.
IN_CHUNKS = ((512, "sp"), (512, "sp"), (2048, "act"), (2048, "sp"),
             (1536, "act"), (1024, "sp"), (512, "sp"))
assert sum(w for w, _ in IN_CHUNKS) == HALF
assert all(w % MM_N == 0 for w, _ in IN_CHUNKS)

_CACHE = {}


def _build_nc(in_chunks=IN_CHUNKS):
    import concourse.bacc as bacc
    import concourse.mybir as mybir
    from concourse.tile import TileContext

    f32 = mybir.dt.float32
    fp8 = mybir.dt.float8e4

    nc = bacc.Bacc("TRN2", target_bir_lowering=False, name="daf_mm")
    xt = nc.dram_tensor("xt", [P, HALF], fp8, kind="ExternalInput")
    mt = nc.dram_tensor("mt", [P, 32], fp8, kind="ExternalInput")
    yt = nc.dram_tensor("yt", [P, OUT_W], fp8, kind="ExternalOutput")

    with TileContext(nc) as tc:
        with (
            tc.tile_pool(name="const", bufs=1) as constp,
            tc.tile_pool(name="io", bufs=1) as iop,
            tc.tile_pool(name="ps", bufs=1, space="PSUM") as psp,
        ):
            # Stationary byte-table: tiny, first dispatch on the ACT ring.
            mt_st = constp.tile([P, 32], fp8, tag="mt")
            nc.scalar.dma_start(mt_st[:], mt[:])

            # All input DMAs dispatched next, alternating rings.
            in_tiles = []   # (tile, col_start, width)
            cs = 0
            for ci, (cw, q) in enumerate(in_chunks):
                xtile = iop.tile([P, cw], fp8, tag=f"x{ci}")
                eng = {"sp": nc.sync, "act": nc.scalar, "gp": nc.gpsimd}[q]
                eng.dma_start(xtile[:], xt[:][:, cs : cs + cw])
                in_tiles.append((xtile, cs, cw))
                cs += cw

            def rhs_window(w):
                col = w * MM_N
                for xtile, xcs, xcw in in_tiles:
                    if xcs <= col and col + MM_N <= xcs + xcw:
                        return xtile[:][:, col - xcs : col - xcs + MM_N]
                raise AssertionError(w)

            ps_tiles = [
                psp.tile([P, MM_N], f32, tag=f"ps{b}", name=f"ps{b}")
                for b in range(N_BANKS)
            ]
            o01 = iop.tile([P, 2 * MM_N], fp8, tag="o01")
            o23 = iop.tile([P, 2 * MM_N], fp8, tag="o23")
            for w in range(N_WIN):
                b, s = w // 4, w % 4
                ps = ps_tiles[b]
                nc.tensor.matmul(
                    ps[:][32 * s : 32 * s + 32, :],
                    mt_st[:],
                    rhs_window(w),
                    # Every matmul is its own "group": start=True clears the
                    # bank's has_written bits (data of other strips persists)
                    # and overwrites this strip; start=False would accumulate
                    # onto stale PSUM state from a previous NEFF execution.
                    start=True,
                    stop=True,
                    tile_position=(0, 32 * s),
                    skip_group_check=True,
                )
                if s == 3:
                    # Banks 0,2 -> DVE; banks 1,3 -> ACT. Each bank pair
                    # shares an SBUF tile; the pair-final ACT copy is
                    # followed by the pair's output DMA (banks 0+1 on the
                    # SP ring, banks 2+3 dispatched from ACT itself so the
                    # last link has no cross-engine hop).
                    otile = o01 if b < 2 else o23
                    dst = otile[:][:, (b % 2) * MM_N : (b % 2 + 1) * MM_N]
                    if b % 2 == 0:
                        nc.vector.tensor_copy(dst, ps[:])
                    else:
                        nc.scalar.copy(dst, ps[:])
                        eng = nc.sync if b == 1 else nc.scalar
                        eng.dma_start(
                            yt[:][:, (b - 1) * MM_N : (b + 1) * MM_N],
                            otile[:],
                        )

    nc.finalize()
    return nc


def _get_nc(in_chunks=IN_CHUNKS):
    key = in_chunks
    if key not in _CACHE:
        _CACHE[key] = _build_nc(in_chunks)
    return _CACHE[key]


def _host_mtab(W: np.ndarray, b: np.ndarray) -> np.ndarray:
    """[128, 32] fp8 byte-table: lanes 0-7 halfA, 8-15 halfB, 16-31 zero."""
    import ml_dtypes

    net = W.astype(np.float32) + b.astype(np.float32)[None, :]   # [V, 2V]
    loc_tok = np.argmax(net[:, :V], axis=1)
    scale_tok = np.argmax(net[:, V:], axis=1)
    tgt = (scale_tok * np.arange(V, dtype=np.int64) + loc_tok) % V
    bt = np.zeros((V, 8), dtype=np.float32)
    nz = np.flatnonzero(scale_tok != 0)
    bt[nz, tgt[nz] // 8] = (1 << (tgt[nz] % 8)).astype(np.float32)
    mt = np.zeros((P, 32), dtype=np.float32)
    mt[:V, :8] = bt
    mt[V:, 8:16] = bt
    return mt.astype(ml_dtypes.float8_e4m3)


def _host_in_maps(inputs: np.ndarray, W: np.ndarray, b: np.ndarray):
    import ml_dtypes

    x8 = inputs.reshape(ROWS, V).astype(ml_dtypes.float8_e4m3)
    # [core, half, row, v] -> [core, half, v, row] -> [core, 128, HALF]
    xt = np.ascontiguousarray(
        x8.reshape(N_CORES, 2, HALF, V).transpose(0, 1, 3, 2)
    ).reshape(N_CORES, P, HALF)
    mt = _host_mtab(W, b)
    return [{"xt": xt[c], "mt": mt} for c in range(N_CORES)]


def _host_gather(results, shape, dtype) -> np.ndarray:
    import ml_dtypes

    yt = np.stack([np.asarray(r["yt"]) for r in results])   # [8, 128, 2048] fp8
    # fp8 -> uint8 integer values (all entries are {0, 1, 2, ..., 128}: exact)
    lut = np.arange(256, dtype=np.uint8).view(ml_dtypes.float8_e4m3)
    lut = lut.astype(np.float32)
    lut[~np.isfinite(lut)] = 0.0
    lut = np.clip(lut, 0, 255).astype(np.uint8)
    ytu = lut[yt.view(np.uint8)]                            # [8, 128, 2048] u8
    v5 = ytu.reshape(N_CORES, 4, 32, N_BANKS, MM_N)         # [c, s, m, b, nl]
    bytes_ = v5[:, :, :16]                                  # [c, s, 16, b, nl]
    bytes_ = bytes_.transpose(0, 2, 3, 1, 4)                # [c, hj, b, s, nl]
    bytes_ = bytes_.reshape(N_CORES, 2, 8, N_WIN, MM_N)     # [c, h, j, w, nl]
    bytes_ = np.ascontiguousarray(
        bytes_.transpose(0, 1, 3, 4, 2)                     # [c, h, w, nl, j]
    ).reshape(N_CORES, 2, HALF, 8)
    bits = np.unpackbits(bytes_, axis=-1, bitorder="little")
    y = bits.reshape(ROWS, V).astype(np.float32)
    return y.reshape(shape).astype(dtype, copy=False)


def kernel(inputs: np.ndarray, W: np.ndarray, b: np.ndarray) -> np.ndarray:
    from concourse import bass_utils

    in_maps = _host_in_maps(np.asarray(inputs), np.asarray(W), np.asarray(b))
    nc = _get_nc()
    res = bass_utils.run_bass_kernel_spmd(nc, in_maps, core_ids=list(range(N_CORES)))
    return _host_gather(res.results, inputs.shape, inputs.dtype)


# revision 5
# speedup vs baseline: 1.0751x; 1.0086x over previous
"""Trainium2 Bass kernel for nn_DiscreteAutoregressiveFlow (sampling, forward).

Math: `inputs` is an exact one-hot [B, L, V] tensor. For a row holding token v
the reference reduces to out_row = one_hot((scale_tok[v]*v + loc_tok[v]) % V),
or the zero row when scale_tok[v] == 0, where loc_tok/scale_tok are argmaxes
of W[v]+b halves (host-precomputable from W/b alone). So the whole flow is a
fixed linear map applied per row, which TensorE evaluates as a matmul against
a host-built table. All products/sums are exact in fp8/f32.

Device pipeline (memory-bound streaming):
  - Host pre-transposes x to xt [128, 8192] fp8 per core: partition = 64*half
    + v, column = row-within-half. One-hots are exact in fp8_e4m3.
  - The table encodes the OUTPUT ONE-HOT AS AN 8-BYTE BITMASK: lhsT[v, j]
    holds float(1 << (u%8)) at j = u//8 for u = cmap[v] (values {1..128},
    exact in fp8e4m3). Each matmul therefore emits [32, 512] (8 byte-lanes x
    2 halves + 16 zero-pad lanes) instead of [128, 512]: 4 matmuls pack one
    PSUM bank at tile_position col offsets 0/32/64/96, so one PSUM->SBUF
    cast-copy drains FOUR matmuls (4 copies total instead of 16) and the
    output DMA shrinks to 256 KB/core. Host np.unpackbits restores one-hots.
  - 16 matmuls of N=512 over a once-loaded stationary table; 8192 PE column
    cycles per core.

Scheduling: each HWDGE dma_start costs ~650ns of SERIAL sequencer time
(DIRECT2D descriptor generation) regardless of size, so DMAs are few and
large, with input dispatch split across both HWDGE rings (SP + ACT) and a
small head chunk so TensorE starts early.
Sharding: pure data parallel over B*L rows, 8 cores, no collectives.
"""

import numpy as np

V = 64
P = 128
N_CORES = 8
B, L = 16, 8192
ROWS = B * L                      # 131072
ROWS_PER_CORE = ROWS // N_CORES   # 16384
HALF = ROWS_PER_CORE // 2         # 8192 columns per core (2 rows per column)
MM_N = 512                        # PSUM bank = 512 f32
N_WIN = HALF // MM_N              # 16 matmul windows
N_BANKS = N_WIN // 4              # 4 windows packed per PSUM bank
OUT_W = N_BANKS * MM_N            # 2048 output columns (byte-packed)

# Input chunks (width, ring), tapered and interleaved across both HWDGE
# rings: small head chunks so TensorE starts as soon as the first completion
# receipt lands, a small tail chunk so the last matmuls wait minimally.
IN_CHUNKS = ((512, "sp"), (512, "sp"), (2048, "act"), (2048, "sp"),
             (1536, "act"), (1024, "sp"), (512, "sp"))
assert sum(w for w, _ in IN_CHUNKS) == HALF
assert all(w % MM_N == 0 for w, _ in IN_CHUNKS)

_CACHE = {}


def _build_nc(in_chunks=IN_CHUNKS):
    import concourse.bacc as bacc
    import concourse.mybir as mybir
    from concourse.tile import TileContext

    f32 = mybir.dt.float32
    fp8 = mybir.dt.float8e4

    nc = bacc.Bacc("TRN2", target_bir_lowering=False, name="daf_mm")
    xt = nc.dram_tensor("xt", [P, HALF], fp8, kind="ExternalInput")
    mt = nc.dram_tensor("mt", [P, 32], fp8, kind="ExternalInput")
    yt = nc.dram_tensor("yt", [P, OUT_W], fp8, kind="ExternalOutput")

    with TileContext(nc) as tc:
        with (
            tc.tile_pool(name="const", bufs=1) as constp,
            tc.tile_pool(name="io", bufs=1) as iop,
            tc.tile_pool(name="ps", bufs=1, space="PSUM") as psp,
        ):
            # Stationary byte-table: tiny, first dispatch on the ACT ring.
            mt_st = constp.tile([P, 32], fp8, tag="mt")
            nc.scalar.dma_start(mt_st[:], mt[:])

            # All input DMAs dispatched next, alternating rings.
            in_tiles = []   # (tile, col_start, width)
            cs = 0
            for ci, (cw, q) in enumerate(in_chunks):
                xtile = iop.tile([P, cw], fp8, tag=f"x{ci}")
                eng = {"sp": nc.sync, "act": nc.scalar, "gp": nc.gpsimd}[q]
                eng.dma_start(xtile[:], xt[:][:, cs : cs + cw])
                in_tiles.append((xtile, cs, cw))
                cs += cw

            def rhs_window(w):
                col = w * MM_N
                for xtile, xcs, xcw in in_tiles:
                    if xcs <= col and col + MM_N <= xcs + xcw:
                        return xtile[:][:, col - xcs : col - xcs + MM_N]
                raise AssertionError(w)

            ps_tiles = [
                psp.tile([P, MM_N], f32, tag=f"ps{b}", name=f"ps{b}")
                for b in range(N_BANKS)
            ]
            o01 = iop.tile([P, 2 * MM_N], fp8, tag="o01")
            o23 = iop.tile([P, 2 * MM_N], fp8, tag="o23")
            for w in range(N_WIN):
                b, s = w // 4, w % 4
                ps = ps_tiles[b]
                nc.tensor.matmul(
                    ps[:][32 * s : 32 * s + 32, :],
                    mt_st[:],
                    rhs_window(w),
                    # Every matmul is its own "group": start=True clears the
                    # bank's has_written bits (data of other strips persists)
                    # and overwrites this strip; start=False would accumulate
                    # onto stale PSUM state from a previous NEFF execution.
                    start=True,
                    stop=True,
                    tile_position=(0, 32 * s),
                    skip_group_check=True,
                )
                if s == 3:
                    # Banks 0,2 -> DVE; banks 1,3 -> ACT. Each bank pair
                    # shares an SBUF tile; the pair-final ACT copy is
                    # followed by the pair's output DMA (banks 0+1 on the
                    # SP ring, banks 2+3 dispatched from ACT itself so the
                    # last link has no cross-engine hop).
                    otile = o01 if b < 2 else o23
                    dst = otile[:][:, (b % 2) * MM_N : (b % 2 + 1) * MM_N]
                    if b % 2 == 0:
                        nc.vector.tensor_copy(dst, ps[:])
                    else:
                        nc.scalar.copy(dst, ps[:])
                        eng = nc.sync if b == 1 else nc.scalar
                        eng.dma_start(
                            yt[:][:, (b - 1) * MM_N : (b + 1) * MM_N],
                            otile[:],
                        )

    nc.finalize()
    return nc


def _get_nc(in_chunks=IN_CHUNKS):
    key = in_chunks
    if key not in _CACHE:
        _CACHE[key] = _build_nc(in_chunks)
    return _CACHE[key]


def _host_mtab(W: np.ndarray, b: np.ndarray) -> np.ndarray:
    """[128, 32] fp8 byte-table: lanes 0-7 halfA, 8-15 halfB, 16-31 zero."""
    import ml_dtypes

    net = W.astype(np.float32) + b.astype(np.float32)[None, :]   # [V, 2V]
    loc_tok = np.argmax(net[:, :V], axis=1)
    scale_tok = np.argmax(net[:, V:], axis=1)
    tgt = (scale_tok * np.arange(V, dtype=np.int64) + loc_tok) % V
    bt = np.zeros((V, 8), dtype=np.float32)
    nz = np.flatnonzero(scale_tok != 0)
    bt[nz, tgt[nz] // 8] = (1 << (tgt[nz] % 8)).astype(np.float32)
    mt = np.zeros((P, 32), dtype=np.float32)
    mt[:V, :8] = bt
    mt[V:, 8:16] = bt
    return mt.astype(ml_dtypes.float8_e4m3)


def _host_in_maps(inputs: np.ndarray, W: np.ndarray, b: np.ndarray):
    import ml_dtypes

    x8 = inputs.reshape(ROWS, V).astype(ml_dtypes.float8_e4m3)
    # [core, half, row, v] -> [core, half, v, row] -> [core, 128, HALF]
    xt = np.ascontiguousarray(
        x8.reshape(N_CORES, 2, HALF, V).transpose(0, 1, 3, 2)
    ).reshape(N_CORES, P, HALF)
    mt = _host_mtab(W, b)
    return [{"xt": xt[c], "mt": mt} for c in range(N_CORES)]


def _host_gather(results, shape, dtype) -> np.ndarray:
    import ml_dtypes

    yt = np.stack([np.asarray(r["yt"]) for r in results])   # [8, 128, 2048] fp8
    # fp8 -> uint8 integer values (all entries are {0, 1, 2, ..., 128}: exact)
    lut = np.arange(256, dtype=np.uint8).view(ml_dtypes.float8_e4m3)
    lut = lut.astype(np.float32)
    lut[~np.isfinite(lut)] = 0.0
    lut = np.clip(lut, 0, 255).astype(np.uint8)
    ytu = lut[yt.view(np.uint8)]                            # [8, 128, 2048] u8
    v5 = ytu.reshape(N_CORES, 4, 32, N_BANKS, MM_N)         # [c, s, m, b, nl]
    bytes_ = v5[:, :, :16]                                  # [c, s, 16, b, nl]
    bytes_ = bytes_.transpose(0, 2, 3, 1, 4)                # [c, hj, b, s, nl]
    bytes_ = bytes_.reshape(N_CORES, 2, 8, N_WIN, MM_N)     # [c, h, j, w, nl]
    bytes_ = np.ascontiguousarray(
        bytes_.transpose(0, 1, 3, 4, 2)                     # [c, h, w, nl, j]
    ).reshape(N_CORES, 2, HALF, 8)
    bits = np.unpackbits(bytes_, axis=-1, bitorder="little")
    y = bits.reshape(ROWS, V).astype(np.float32)
    return y.reshape(shape).astype(dtype, copy=False)


def kernel(inputs: np.ndarray, W: np.ndarray, b: np.ndarray) -> np.ndarray:
    from concourse import bass_utils

    in_maps = _host_in_maps(np.asarray(inputs), np.asarray(W), np.asarray(b))
    nc = _get_nc()
    res = bass_utils.run_bass_kernel_spmd(nc, in_maps, core_ids=list(range(N_CORES)))
    return _host_gather(res.results, inputs.shape, inputs.dtype)
